# revision 1
# baseline (speedup 1.0000x reference)
"""Trainium2 Bass kernel for nn_BaselineModel (4-layer SiLU-attention transformer).

Sharding: 8 cores = 2 batches x 4 token-chunks. Projections, norms, gating and
out-proj run token-local; attention is head-parallel via an AllToAll reshard
(Ulysses-style) within each batch group of 4 cores. Matmul operands are bf16
(f32 psum accumulation); the residual stream stays f32 on-chip.

Self-contained: hardcodes shapes/sharding; needs only numpy/ml_dtypes/concourse.
"""
import numpy as np
import ml_dtypes

import concourse.bass as bass
import concourse.mybir as mybir
import concourse.tile as tile
from concourse import bacc
from concourse.bass_utils import run_bass_kernel_spmd

F32 = mybir.dt.float32
BF16 = mybir.dt.bfloat16
I32 = mybir.dt.int32
AF = mybir.ActivationFunctionType
ALU = mybir.AluOpType
P = 128

N_CORES = 8
GROUPS = [[0, 1, 2, 3, 4, 5, 6, 7]]

import os
FENCE_SCORE = os.environ.get("FENCE_SCORE", "1") == "1"
FENCE_SSQ = os.environ.get("FENCE_SSQ", "1") == "1"
FENCE_DVE = os.environ.get("FENCE_DVE", "1") == "1"

CW_C1, CW_C2, CW_C3 = 6.283203125, -1.7762184143066406e-05, -5.563627070159782e-08
MAGIC = 12582912.0
EPS = 1e-6
ROPE_BASE = 10000.0


class Cfg:
    def __init__(self, B=2, S=2048, D=1024, H=16, L=4):
        self.B, self.S, self.D, self.H, self.L = B, S, D, H, L
        self.HD = D // H
        assert self.HD == 64
        self.T = B * S // N_CORES            # tokens per core (Tb from each batch)
        self.Tb = self.T // B
        assert self.T * N_CORES == B * S and self.Tb % P == 0
        self.DC = D // P                     # d-chunks of 128
        assert D % 512 == 0
        self.QT = min(512, S)                # attention q-tile width
        assert S % self.QT == 0 and self.QT % self.Tb == 0 or self.Tb % self.QT == 0
        self.NQ = S // self.QT               # global q tiles
        self.HPC = H // 8                    # head-pairs per core
        assert H % 8 == 0
        self.D4 = D // 4                     # head-quad rows per a2a shard
        self.OSH = self.DC // 4              # o-chunks per head-quad
        self.VW = min(512, D)
        self.NVW = D // self.VW
        self.NCT = 4 * D // P                # weight c-tiles (128 wide)


DEFAULT_CFG = Cfg()


def build(cfg=DEFAULT_CFG, n_layers=None):
    B, S, D, H = cfg.B, cfg.S, cfg.D, cfg.H
    L = cfg.L if n_layers is None else n_layers
    T, DC, QT, NQ, HPC = cfg.T, cfg.DC, cfg.QT, cfg.NQ, cfg.HPC
    HD, D4, OSH, VW, NVW = cfg.HD, cfg.D4, cfg.OSH, cfg.VW, cfg.NVW
    Tb = cfg.Tb
    TPb = Tb // P
    HD2 = HD // 2
    scale = float(1.0 / np.sqrt(HD))
    TP = T // P                              # token chunks of 128

    nc = bacc.Bacc("TRN2", target_bir_lowering=False, debug=False,
                   num_devices=N_CORES)
    prev_cc = [None]

    def chain_cc(cc):
        # pin collective issue order (the Tile scheduler otherwise reorders
        # them, serializing a later collective ahead of an earlier one)
        if prev_cc[0] is not None:
            bass._add_dep_helper(cc.ins, prev_cc[0].ins, sync=True,
                                 reason="collective issue order")
        prev_cc[0] = cc
        return cc

    def _raw(h):
        return h.ins if hasattr(h, "ins") else h

    def fence(later, earliers):
        # forced ordering: keep same-queue instructions in ready-time order so
        # queue-head waits don't block earlier-ready work (sync=True: the
        # scheduler ignores nosync edges)
        for e in earliers:
            if e is not None and later is not None:
                bass._add_dep_helper(_raw(later), _raw(e), sync=True,
                                     reason="queue order fence")

    # ---------------- parameters ----------------
    xT = nc.declare_dram_parameter("xT", [P, DC, T], F32, isOutput=False)
    td = nc.declare_dram_parameter("td", [1, T], F32, isOutput=False)
    pid = nc.declare_dram_parameter("pid", [1, T], I32, isOutput=False)
    invf = nc.declare_dram_parameter("invf", [HD2, 1], F32, isOutput=False)
    # host pre-tiled weights: [L, NCT, DC, P, 128] / [L, D//P, DC, P, 128]
    uvqk_w = nc.declare_dram_parameter("uvqk_w", [L, cfg.NCT, P, DC, P], BF16, isOutput=False)
    out_w = nc.declare_dram_parameter("out_w", [L, DC, P, DC, P], BF16, isOutput=False)
    uvqk_b = nc.declare_dram_parameter("uvqk_b", [L, 4 * D], F32, isOutput=False)
    gate_w = nc.declare_dram_parameter("gate_w", [L, D], F32, isOutput=False)
    out_b = nc.declare_dram_parameter("out_b", [L, D], F32, isOutput=False)
    in_nw = nc.declare_dram_parameter("in_nw", [D], F32, isOutput=False)
    last_nw = nc.declare_dram_parameter("last_nw", [D], F32, isOutput=False)
    out_ext = nc.declare_dram_parameter("out", [DC, P, T], F32, isOutput=True)

    with tile.TileContext(nc) as tc:
        with (
            tc.tile_pool(name="const", bufs=1) as cpool,
            tc.tile_pool(name="persist", bufs=1) as ppool,
            tc.tile_pool(name="setup", bufs=1) as spool,
            tc.tile_pool(name="big", bufs=1) as bpool,
            tc.tile_pool(name="wpool", bufs=int(os.environ.get("WPOOL", "5"))) as wpool,
            tc.tile_pool(name="wvpool", bufs=int(os.environ.get("WVP", "2"))) as wvpool,
            tc.tile_pool(name="work", bufs=2) as work,
            tc.tile_pool(name="wtile", bufs=int(os.environ.get("WTL", "3"))) as wtl,
            tc.tile_pool(name="kv", bufs=int(os.environ.get("KVB", "1"))) as kvpool,
            tc.tile_pool(name="mmps", bufs=2, space="PSUM") as mmps,
            tc.tile_pool(name="sps", bufs=2, space="PSUM") as sps,
            tc.tile_pool(name="aps", bufs=1, space="PSUM") as aps,
            tc.tile_pool(name="ssps", bufs=1, space="PSUM") as ssps,
            tc.tile_pool(name="dram", bufs=1, space="DRAM") as dram,
        ):
            # ---------------- constants ----------------
            ones_bf = cpool.tile([P, 1], BF16)
            nc.vector.memset(ones_bf[:], 1.0)
            eps_t = cpool.tile([P, 1], F32)
            nc.vector.memset(eps_t[:], EPS)

            b_u = ppool.tile([P, L, DC], F32)
            b_q = ppool.tile([P, L, DC], F32)
            b_k = ppool.tile([P, L, DC], F32)
            g_w = ppool.tile([P, L, DC], F32)
            o_b = ppool.tile([P, L, DC], F32)
            for ll in range(L):
                nc.scalar.dma_start(b_u[:, ll, :], uvqk_b[ll, 0:D].rearrange("(o p) -> p o", p=P))
                nc.scalar.dma_start(b_q[:, ll, :], uvqk_b[ll, 2 * D:3 * D].rearrange("(o p) -> p o", p=P))
                nc.scalar.dma_start(b_k[:, ll, :], uvqk_b[ll, 3 * D:4 * D].rearrange("(o p) -> p o", p=P))
                nc.scalar.dma_start(g_w[:, ll, :], gate_w[ll].rearrange("(o p) -> p o", p=P))
                nc.scalar.dma_start(o_b[:, ll, :], out_b[ll].rearrange("(o p) -> p o", p=P))
            in_nw_t = ppool.tile([P, DC], F32)
            last_nw_t = ppool.tile([P, DC], F32)
            nc.scalar.dma_start(in_nw_t[:], in_nw.rearrange("(o p) -> p o", p=P))
            nc.scalar.dma_start(last_nw_t[:], last_nw.rearrange("(o p) -> p o", p=P))
            bv_row = ppool.tile([1, L, D], F32)
            nc.scalar.dma_start(bv_row[:], uvqk_b[None, :, D:2 * D])

            # ---------------- rope tables (once) ----------------
            td_t = spool.tile([1, T], F32)
            pid_t = spool.tile([1, T], I32)
            invf_t = cpool.tile([HD2, 1], F32)
            nc.sync.dma_start(td_t[:], td[:, :])
            nc.sync.dma_start(pid_t[:], pid[:, :])
            nc.sync.dma_start(invf_t[:], invf[:, :])
            pid_f = spool.tile([1, T], F32)
            nc.vector.tensor_copy(pid_f[:], pid_t[:])
            ln_t = spool.tile([1, T], F32)
            nc.scalar.activation(ln_t[:], td_t[:], AF.Ln, bias=1.0)
            pos_t = spool.tile([1, T], F32)
            nc.vector.scalar_tensor_tensor(pos_t[:], ln_t[:], 0.1, pid_f[:],
                                           op0=ALU.mult, op1=ALU.add)
            pos_bc = spool.tile([HD2, T], F32)
            nc.gpsimd.partition_broadcast(pos_bc[:], pos_t[:], channels=HD2)
            freqs = spool.tile([HD2, T], F32)
            nc.vector.tensor_scalar(freqs[:], pos_bc[:], invf_t[:], None, op0=ALU.mult)
            k_r = spool.tile([HD2, T], F32)
            nc.vector.tensor_scalar(k_r[:], freqs[:], float(1.0 / (2 * np.pi)), None, op0=ALU.mult)
            nc.vector.tensor_scalar(k_r[:], k_r[:], MAGIC, MAGIC, op0=ALU.add, op1=ALU.subtract)
            sin_arg = spool.tile([HD2, T], F32)
            nc.vector.cody_waite_cascade(sin_arg[:], freqs[:], k_r[:], CW_C1, CW_C2, CW_C3)
            cos_arg = spool.tile([HD2, T], F32)
            nc.vector.add_range_wrap(cos_arg[:], sin_arg[:], float(np.pi / 2),
                                     float(np.pi), float(2 * np.pi))
            cs32 = spool.tile([HD2, T], F32)
            sn32 = spool.tile([HD2, T], F32)
            nc.scalar.activation(cs32[:], cos_arg[:], AF.Sin)
            nc.scalar.activation(sn32[:], sin_arg[:], AF.Sin)
            cos2 = ppool.tile([P, T], BF16)
            sinneg = ppool.tile([P, T], BF16)
            for hh in range(2):
                b0 = hh * 64
                nc.vector.tensor_copy(cos2[b0:b0 + HD2, :], cs32[:])
                nc.vector.tensor_copy(cos2[b0 + HD2:b0 + HD, :], cs32[:])
                nc.vector.tensor_scalar(sinneg[b0:b0 + HD2, :], sn32[:], -1.0, None, op0=ALU.mult)
                nc.vector.tensor_copy(sinneg[b0 + HD2:b0 + HD, :], sn32[:])

            # causal masks for the 4 diagonal-crossing j-tile offsets
            # mask2[p, mi, x] = 1 if x - p - 128*mi >= 0 else 0
            mask2 = ppool.tile([P, 4, QT], BF16)
            nc.vector.memset(mask2[:], 1.0)
            for mi in range(4):
                nc.gpsimd.affine_select(
                    mask2[:, mi, :], mask2[:, mi, :],
                    pattern=[[1, QT]],
                    compare_op=ALU.is_ge, fill=0.0,
                    base=-128 * mi, channel_multiplier=-1)

            # ---------------- helpers ----------------
            def rms_scale_from(src_tile):
                """[1,T] inv-rms over D of a [P, DC, T] bf16 tile -> bcast [P, T] f32."""
                ss_ps = ssps.tile([1, T], F32, tag="ssq")
                for o in range(DC):
                    sq = work.tile([P, T], BF16, tag="sq")
                    nc.vector.tensor_tensor(sq[:], src_tile[:, o, :], src_tile[:, o, :], op=ALU.mult)
                    nc.tensor.matmul(ss_ps[:], ones_bf[:], sq[:],
                                     start=(o == 0), stop=(o == DC - 1))
                lnm = work.tile([1, T], F32, tag="lnm")
                nc.scalar.activation(lnm[:], ss_ps[:], AF.Ln, scale=1.0 / D, bias=eps_t[:1])
                sc = work.tile([1, T], F32, tag="rms")
                nc.scalar.activation(sc[:], lnm[:], AF.Exp, scale=-0.5)
                sc_bc = work.tile([P, T], F32, tag="rmsbc")
                nc.gpsimd.partition_broadcast(sc_bc[:], sc[:], channels=P)
                return sc_bc

            # ---------------- h0 = rms_norm(x, in_norm_w) ----------------
            h = ppool.tile([P, DC, T], F32)
            h_bf = ppool.tile([P, DC, T], BF16)
            nc.sync.dma_start(h[:], xT[:, :, :])
            ss_ps = ssps.tile([1, T], F32, tag="ssq")
            for o in range(DC):
                sq = work.tile([P, T], BF16, tag="sq")
                nc.vector.tensor_tensor(sq[:], h[:, o, :], h[:, o, :], op=ALU.mult)
                nc.tensor.matmul(ss_ps[:], ones_bf[:], sq[:],
                                 start=(o == 0), stop=(o == DC - 1))
            lnm = work.tile([1, T], F32, tag="lnm")
            nc.scalar.activation(lnm[:], ss_ps[:], AF.Ln, scale=1.0 / D, bias=eps_t[:1])
            sc0 = work.tile([1, T], F32, tag="rms")
            nc.scalar.activation(sc0[:], lnm[:], AF.Exp, scale=-0.5)
            sc_bc = work.tile([P, T], F32, tag="rmsbc")
            nc.gpsimd.partition_broadcast(sc_bc[:], sc0[:], channels=P)
            for o in range(DC):
                nc.vector.scalar_tensor_tensor(h[:, o, :], h[:, o, :],
                                               in_nw_t[:, o:o + 1], sc_bc[:],
                                               op0=ALU.mult, op1=ALU.mult)
                nc.vector.tensor_copy(h_bf[:, o, :], h[:, o, :])

            # ---------------- a2a dram buffers (flat shards, per half) ----------------
            RB = P * Tb
            SH1 = 3 * RB
            a2a_in1 = [dram.tile([8, SH1], BF16, name=f"a2a_in1_{i}") for i in range(2)]
            a2a_out1 = [dram.tile([8, SH1], BF16, name=f"a2a_out1_{i}") for i in range(2)]
            a2a_in2 = [dram.tile([8, RB], BF16, name=f"a2a_in2_{i}") for i in range(2)]
            a2a_out2 = [dram.tile([8, RB], BF16, name=f"a2a_out2_{i}") for i in range(2)]

            for l in range(L):
                # ---------------- uvqk projection + a2a1, per head-pair half ----------
                uT = ppool.tile([P, DC, T], BF16, tag="uT")
                qT = ppool.tile([P, 4, 2, T], BF16, tag="qT")   # [p, quad, half, t]
                kT = ppool.tile([P, 4, 2, T], BF16, tag="kT")
                # v token-major, a2a-payload order: [p, half, quad, ba, tb-chunk, c]
                vtm = ppool.tile([P, 2, 4, 2, TPb, P], BF16, tag="vtm")
                bv_bc = spool.tile([P, D], F32, tag="bvbc")
                nc.gpsimd.partition_broadcast(bv_bc[:], bv_row[:, l, :], channels=P)

                uvqk_dve = []
                uvqk_mm = []
                for hf in range(2):
                    # q,k chunks of this half (+rope), interleaved per quad
                    for qd in range(4):
                        ct = 2 * qd + hf
                        for cbase, dest, bias in ((2 * DC, qT, b_q), (3 * DC, kT, b_k)):
                            wt = wpool.tile([P, DC, P], BF16, tag="w_uqk")
                            nc.sync.dma_start(wt[:], uvqk_w[l, cbase + ct])
                            ps = mmps.tile([P, T], F32, tag="mm")
                            for dc in range(DC):
                                uvqk_mm.append(nc.tensor.matmul(
                                    ps[:], wt[:, dc, :], h_bf[:, dc, :],
                                    start=(dc == 0), stop=(dc == DC - 1)))
                            dv = dest[:, qd, hf, :]
                            nc.vector.tensor_scalar(dv, ps[:],
                                                    bias[:, l, ct:ct + 1], None, op0=ALU.add)
                            # rope in place
                            sw = work.tile([P, T], BF16, tag="swap")
                            for hh in range(2):
                                b0 = hh * 64
                                nc.vector.tensor_copy(sw[b0:b0 + HD2, :],
                                                      dest[b0 + HD2:b0 + HD, qd, hf, :])
                                nc.vector.tensor_copy(sw[b0 + HD2:b0 + HD, :],
                                                      dest[b0:b0 + HD2, qd, hf, :])
                            t1 = work.tile([P, T], BF16, tag="ropet1")
                            nc.vector.tensor_tensor(t1[:], dv, cos2[:], op=ALU.mult)
                            t2 = work.tile([P, T], BF16, tag="ropet2")
                            nc.vector.tensor_tensor(t2[:], sw[:], sinneg[:], op=ALU.mult)
                            uvqk_dve.append(
                                nc.vector.tensor_tensor(dv, t1[:], t2[:], op=ALU.add))
                    # v half: host-permuted weight tiles DC+4*hf .. DC+4*hf+3
                    vt_w = wvpool.tile([P, DC, VW], BF16, tag="w_v")
                    nc.sync.dma_start(
                        vt_w[:].rearrange("p d (a c) -> p d a c", c=P),
                        uvqk_w[l, DC + 4 * hf:DC + 4 * (hf + 1)]
                        .rearrange("a p d c -> p d a c"))
                    for tt in range(TP):
                        ba, aa = divmod(tt, TPb)
                        ps = mmps.tile([P, VW], F32, tag="mm")
                        for dc in range(DC):
                            uvqk_mm.append(nc.tensor.matmul(
                                ps[:], h_bf[:, dc, tt * P:(tt + 1) * P],
                                vt_w[:, dc, :],
                                start=(dc == 0), stop=(dc == DC - 1)))
                        uvqk_dve.append(nc.vector.tensor_tensor(
                            vtm[:, hf, :, ba, aa, :],
                            ps[:].rearrange("p (q c) -> p q c", c=P),
                            bv_bc[:, hf * VW:(hf + 1) * VW]
                            .rearrange("p (q c) -> p q c", c=P), op=ALU.add))
                    # pack (2 DMAs per tensor) + collective for this half
                    for ba in range(2):
                        jsl = slice(ba * 4, (ba + 1) * 4)
                        tsl = slice(ba * Tb, (ba + 1) * Tb)
                        nc.gpsimd.dma_start(
                            a2a_in1[hf][jsl, 0:RB]
                            .rearrange("j (p t) -> p j t", p=P),
                            qT[:, :, hf, tsl])
                        nc.gpsimd.dma_start(
                            a2a_in1[hf][jsl, RB:2 * RB]
                            .rearrange("j (p t) -> p j t", p=P),
                            kT[:, :, hf, tsl])
                        nc.gpsimd.dma_start(
                            a2a_in1[hf][jsl, 2 * RB:3 * RB]
                            .rearrange("j (p a c) -> p j a c", p=P, c=P),
                            vtm[:, hf, :, ba, :, :])
                    chain_cc(nc.gpsimd.collective_compute(
                        "AllToAll", ALU.bypass, replica_groups=GROUPS,
                        ins=[a2a_in1[hf][:].opt()], outs=[a2a_out1[hf][:].opt()]))

                # u projection (PE fills while a2a1 flies; needed only at gating)
                for ct in range(DC):
                    wt = wpool.tile([P, DC, P], BF16, tag="w_uqk")
                    nc.sync.dma_start(wt[:], uvqk_w[l, ct])
                    ps = mmps.tile([P, T], F32, tag="mm")
                    for dc in range(DC):
                        uvqk_mm.append(nc.tensor.matmul(
                            ps[:], wt[:, dc, :], h_bf[:, dc, :],
                            start=(dc == 0), stop=(dc == DC - 1)))
                    nc.scalar.activation(uT[:, ct, :], ps[:], AF.Silu,
                                         bias=b_u[:, l, ct:ct + 1])

                # ---------------- attention (head-parallel, per half) ----------------
                # at2[p, hf, quad, t]: d-chunk (2*quad+hf) of attn (gating input)
                at2 = bpool.tile([P, 2, 4, T], BF16, tag="at")
                prev_half_dve = None
                prev_half_mm = None
                last_attn_mm = None
                last_attnT_copy = None
                for hf in range(2):
                    q_hp = kvpool.tile([P, S // Tb, Tb], BF16, tag="q_hp")
                    k_hp = kvpool.tile([P, S // Tb, Tb], BF16, tag="k_hp")
                    v_hp = kvpool.tile([P, S // P, P], BF16, tag="v_hp")
                    nc.scalar.dma_start(
                        q_hp[:],
                        a2a_out1[hf][:, 0:RB].rearrange("r (p t) -> p r t", p=P))
                    nc.scalar.dma_start(
                        k_hp[:],
                        a2a_out1[hf][:, RB:2 * RB].rearrange("r (p t) -> p r t", p=P))
                    nc.sync.dma_start(
                        v_hp[:].rearrange("p (r a) c -> p r a c", a=TPb),
                        a2a_out1[hf][:, 2 * RB:3 * RB]
                        .rearrange("r (p a c) -> p r a c", p=P, c=P))
                    attnT = work.tile([P, NQ, QT], BF16, tag="attnT")
                    k_fl = k_hp[:].rearrange("p a t -> p (a t)")
                    q_fl = q_hp[:].rearrange("p a t -> p (a t)")
                    # software-pipelined: emit scores LA stages ahead of the av
                    # matmuls so the in-order PE queue never stalls on the
                    # silu/mask round-trip of the current stage
                    stages = [(qt, ji, (qt + 1) * QT // P)
                              for qt in range(NQ)
                              for ji in range((qt + 1) * QT // P)]
                    LA = int(os.environ.get("LA", "2"))
                    w_ts = {}
                    a_ps_map = {}

                    def emit_front(i, hf):
                        nonlocal prev_half_mm, uvqk_mm, prev_half_dve
                        nonlocal uvqk_dve, last_attn_mm
                        qt, ji, njt = stages[i]
                        s_ps = sps.tile([P, 1024], F32, tag="sps")
                        for hh in range(2):
                            smm = nc.tensor.matmul(
                                s_ps[:, hh * 512:hh * 512 + QT],
                                k_fl[hh * 64:(hh + 1) * 64, ji * P:(ji + 1) * P],
                                q_fl[hh * 64:(hh + 1) * 64, qt * QT:(qt + 1) * QT],
                                start=True, stop=True,
                                tile_position=(hh * 64, 0))
                            if prev_half_mm is not None:
                                # attn-h1 PE work strictly after attn-h0's
                                fence(smm, [prev_half_mm])
                                prev_half_mm = None
                            if uvqk_mm and FENCE_SCORE:
                                fence(smm, uvqk_mm)
                            if uvqk_mm:
                                uvqk_mm = []
                            last_attn_mm = smm
                        w_t = wtl.tile([P, 2, QT], BF16, tag="wt", name="w_t")
                        sview = s_ps[:].rearrange("p (h x) -> p h x", h=2)[:, :, 0:QT]
                        nc.scalar.activation(w_t[:], sview, AF.Silu, scale=scale)
                        dq = qt * QT - ji * P
                        if dq < P:  # diagonal-crossing j tile: mask on DVE
                            for hh in range(2):
                                mi = nc.vector.tensor_tensor(
                                    w_t[:, hh, :], w_t[:, hh, :],
                                    mask2[:, -dq // 128, :], op=ALU.mult)
                                if prev_half_dve is not None:
                                    fence(mi, [prev_half_dve])
                                    prev_half_dve = None
                                if uvqk_dve and FENCE_DVE:
                                    fence(mi, uvqk_dve)
                                if uvqk_dve:
                                    uvqk_dve = []
                        w_ts[i] = w_t

                    def emit_back(i, hf):
                        nonlocal last_attn_mm, prev_half_dve, last_attnT_copy
                        qt, ji, njt = stages[i]
                        if ji == 0:
                            a_ps_map[qt] = aps.tile([P, QT], F32, tag="aps",
                                                    name="a_ps")
                        a_ps = a_ps_map[qt]
                        w_t = w_ts.pop(i)
                        for hh in range(2):
                            last_attn_mm = nc.tensor.matmul(
                                a_ps[hh * 64:(hh + 1) * 64, :],
                                v_hp[:, ji, hh * 64:(hh + 1) * 64],
                                w_t[:, hh, :],
                                start=(ji == 0), stop=(ji == njt - 1),
                                tile_position=(0, hh * 64),
                                skip_group_check=True)
                        if ji == njt - 1:
                            atc = nc.vector.tensor_copy(attnT[:, qt, :], a_ps[:])
                            nc.sync.dma_start(
                                a2a_in2[hf][2 * qt:2 * (qt + 1), :]
                                .rearrange("r (p t) -> p r t", p=P),
                                attnT[:, qt, :].rearrange("p (r t) -> p r t", t=Tb))
                            if qt == NQ - 1:
                                if hf == 0:
                                    prev_half_dve = atc
                                else:
                                    last_attnT_copy = atc

                    for i in range(len(stages)):
                        emit_front(i, hf)
                        if i >= LA:
                            emit_back(i - LA, hf)
                    for i in range(len(stages) - LA, len(stages)):
                        emit_back(i, hf)
                    chain_cc(nc.gpsimd.collective_compute(
                        "AllToAll", ALU.bypass, replica_groups=GROUPS,
                        ins=[a2a_in2[hf][:].opt()], outs=[a2a_out2[hf][:].opt()]))
                    if hf == 0:
                        prev_half_mm = last_attn_mm

                # ---------------- gated rms + out proj (token-local) ----------------
                for hf2 in range(2):
                    for ba in range(2):
                        nc.scalar.dma_start(
                            at2[:, hf2, :, ba * Tb:(ba + 1) * Tb],
                            a2a_out2[hf2][ba * 4:(ba + 1) * 4]
                            .rearrange("r (p t) -> p r t", p=P))
                # inv-rms over D (per token) from at2; ssq on PE via ones-matmul
                # (half-0 chunks first: they arrive one collective earlier)
                oorder = [0, 2, 4, 6, 1, 3, 5, 7]
                ss_ps = ssps.tile([1, T], F32, tag="ssq")
                for i, o in enumerate(oorder):
                    sq = work.tile([P, T], BF16, tag="sq")
                    src = at2[:, o % 2, o // 2, :]
                    sqi = nc.vector.tensor_tensor(sq[:], src, src, op=ALU.mult)
                    if i == 0:
                        fence(sqi, [last_attnT_copy])
                    qmm = nc.tensor.matmul(ss_ps[:], ones_bf[:], sq[:],
                                           start=(i == 0), stop=(i == DC - 1))
                    if i == 0 and FENCE_SSQ:
                        fence(qmm, [last_attn_mm])
                lnm = work.tile([1, T], F32, tag="lnm")
                nc.scalar.activation(lnm[:], ss_ps[:], AF.Ln, scale=1.0 / D, bias=eps_t[:1])
                sc = work.tile([1, T], F32, tag="rms")
                nc.scalar.activation(sc[:], lnm[:], AF.Exp, scale=-0.5)
                sc_bc = work.tile([P, T], F32, tag="rmsbc")
                nc.gpsimd.partition_broadcast(sc_bc[:], sc[:], channels=P)
                # unscaled gate in place: at2 <- attn * gate_w * u (inv-rms folded
                # into the psum result after the out-proj matmul)
                for o in oorder:
                    nc.vector.scalar_tensor_tensor(at2[:, o % 2, o // 2, :],
                                                   at2[:, o % 2, o // 2, :],
                                                   g_w[:, l, o:o + 1], uT[:, o, :],
                                                   op0=ALU.mult, op1=ALU.mult)
                for et in range(DC):
                    wt = wpool.tile([P, DC, P], BF16, tag="w_o")
                    nc.sync.dma_start(wt[:], out_w[l, et])
                    ps = mmps.tile([P, T], F32, tag="mm")
                    for i, dc in enumerate(oorder):
                        nc.tensor.matmul(ps[:], wt[:, dc, :],
                                         at2[:, dc % 2, dc // 2, :],
                                         start=(i == 0), stop=(i == DC - 1))
                    otmp = work.tile([P, T], F32, tag="xchunk")
                    nc.vector.tensor_tensor(otmp[:], ps[:], sc_bc[:], op=ALU.mult)
                    nc.vector.scalar_tensor_tensor(h[:, et, :], otmp[:], o_b[:, l, et:et + 1],
                                                   h[:, et, :], op0=ALU.add, op1=ALU.add)
                    nc.vector.tensor_copy(h_bf[:, et, :], h[:, et, :])

            # ---------------- final norm ----------------
            sc_bc = rms_scale_from(h_bf)
            for o in range(DC):
                of = work.tile([P, T], F32, tag="of")
                nc.vector.scalar_tensor_tensor(of[:], h[:, o, :], last_nw_t[:, o:o + 1],
                                               sc_bc[:], op0=ALU.mult, op1=ALU.mult)
                nc.sync.dma_start(out_ext[o, :, :], of[:])

    nc.compile()
    return nc


def shard_inputs(inputs, cfg=DEFAULT_CFG):
    B, S, D, H, L = cfg.B, cfg.S, cfg.D, cfg.H, cfg.L
    T, DC, P_ = cfg.T, cfg.DC, P
    x = np.asarray(inputs["x"], dtype=np.float32)
    tdel = np.asarray(inputs["time_deltas"], dtype=np.float32)
    pids = np.asarray(inputs["position_ids"]).astype(np.int32)
    bf = ml_dtypes.bfloat16
    # weights pre-tiled: [L, NCT, DC, P, 128]; v col-tiles permuted so each
    # head-pair half's 512 cols are contiguous: tile DC+4*hf+hq = heads (hq,hf)
    uw = np.asarray(inputs["uvqk_w"], dtype=np.float32).astype(bf)
    uw = uw.reshape(L, DC, P_, cfg.NCT, P_).transpose(0, 3, 2, 1, 4)
    vperm = [8 + 2 * hq + hf for hf in (0, 1) for hq in range(4)]
    tile_order = list(range(8)) + vperm + list(range(16, 32))
    uw = np.ascontiguousarray(uw[:, tile_order])
    ow = np.asarray(inputs["out_w"], dtype=np.float32).astype(bf)
    ow = np.ascontiguousarray(
        ow.reshape(L, DC, P_, DC, P_).transpose(0, 3, 2, 1, 4))
    ub = np.asarray(inputs["uvqk_b"], dtype=np.float32).copy()
    vdims = np.concatenate([np.arange(p * P_, (p + 1) * P_) for p in vperm]) - 8 * P_
    ub[:, D:2 * D] = ub[:, D + vdims]
    ub = np.ascontiguousarray(ub)
    gw = np.ascontiguousarray(np.asarray(inputs["gate_w"], dtype=np.float32))
    ob = np.ascontiguousarray(np.asarray(inputs["out_b"], dtype=np.float32))
    inw = np.ascontiguousarray(np.asarray(inputs["in_norm_w"], dtype=np.float32))
    lnw = np.ascontiguousarray(np.asarray(inputs["last_norm_w"], dtype=np.float32))
    invf = (1.0 / (ROPE_BASE ** (np.arange(0, cfg.HD, 2, dtype=np.float32) / cfg.HD))
            ).astype(np.float32).reshape(-1, 1)
    Tb = cfg.Tb
    in_maps = []
    for c in range(N_CORES):
        sl = slice(c * Tb, (c + 1) * Tb)
        xc = np.concatenate([x[b, sl, :] for b in range(B)], axis=0)  # [T, D]
        xTc = np.ascontiguousarray(
            xc.T.reshape(DC, P_, T).transpose(1, 0, 2))  # [P, DC, T]
        tdc = np.concatenate([tdel[b, sl] for b in range(B)])
        pidc = np.concatenate([pids[b, sl] for b in range(B)])
        in_maps.append({
            "xT": xTc,
            "td": np.ascontiguousarray(tdc.reshape(1, T)),
            "pid": np.ascontiguousarray(pidc.reshape(1, T)),
            "invf": invf,
            "uvqk_w": uw, "uvqk_b": ub, "gate_w": gw,
            "out_w": ow, "out_b": ob, "in_nw": inw, "last_nw": lnw,
        })
    return in_maps


def unshard_output(results, cfg=DEFAULT_CFG):
    B, S, D, T, Tb = cfg.B, cfg.S, cfg.D, cfg.T, cfg.Tb
    out = np.empty((B, S, D), dtype=np.float32)
    for c in range(N_CORES):
        oc = np.asarray(results[c]["out"], dtype=np.float32).reshape(D, T).T  # [T, D]
        for b in range(B):
            out[b, c * Tb:(c + 1) * Tb, :] = oc[b * Tb:(b + 1) * Tb]
    return out


_NC_CACHE = {}


def _get_nc(cfg=DEFAULT_CFG):
    key = (cfg.B, cfg.S, cfg.D, cfg.H, cfg.L)
    if key not in _NC_CACHE:
        _NC_CACHE[key] = build(cfg)
    return _NC_CACHE[key]


def kernel(**inputs):
    cfg = DEFAULT_CFG
    nc = _get_nc(cfg)
    in_maps = shard_inputs(inputs, cfg)
    r = run_bass_kernel_spmd(nc, in_maps, core_ids=list(range(N_CORES)), trace=False)
    return unshard_output(r.results, cfg)



# revision 57
# speedup vs baseline: 1.1694x; 1.1694x over previous
"""Trainium2 Bass kernel for nn_BaselineModel (4-layer SiLU-attention transformer).

Sharding: 8 cores = 2 batches x 4 token-chunks. Projections, norms, gating and
out-proj run token-local; attention is head-parallel via an AllToAll reshard
(Ulysses-style) across the 8 cores, two head-pair halves per layer so each
half's collective hides under the other half's compute.

Precision: bf16 matmuls/wires except two fp8e4m3 spots chosen by an error
ablation against the 2e-2 gate: the V payload of the first all-to-all and the
silu'd score weights - together they enable a full-width DoubleRow AV matmul
(zero-padded interleaved V, 2 contraction tiles per pass). Residual stream
stays f32 on-chip.

Schedule highlights: rope is applied on the RECEIVER side of the all-to-all
(per 256-token block, on the otherwise-idle vector engine under the Act-bound
silu stream), so the projection->collective path has no DVE work; q/k bias-adds
ride the Act engine; a single 3-deep [P,1024] PSUM pool serves every matmul so
the score pipeline never stalls on bank reuse; diagonal causal masks run as
affine_selects on the Pool engine; the u projection and out-proj fill collective
flight time; explicit fences keep waits off engine-queue heads.

Self-contained: hardcodes shapes/sharding; needs only numpy/ml_dtypes/concourse.
"""
import numpy as np
import ml_dtypes

import concourse.bass as bass
import concourse.mybir as mybir
import concourse.tile as tile
from concourse import bacc
from concourse.bass_utils import run_bass_kernel_spmd

F32 = mybir.dt.float32
BF16 = mybir.dt.bfloat16
FP8 = mybir.dt.float8e4
I32 = mybir.dt.int32
AF = mybir.ActivationFunctionType
ALU = mybir.AluOpType
DR = mybir.MatmulPerfMode.DoubleRow
P = 128

N_CORES = 8
GROUPS = [[0, 1, 2, 3, 4, 5, 6, 7]]

import os
FENCE_SCORE = os.environ.get("FENCE_SCORE", "1") == "1"
FENCE_SSQ = os.environ.get("FENCE_SSQ", "1") == "1"
FENCE_DVE = os.environ.get("FENCE_DVE", "1") == "1"

CW_C1, CW_C2, CW_C3 = 6.283203125, -1.7762184143066406e-05, -5.563627070159782e-08
MAGIC = 12582912.0
EPS = 1e-6
ROPE_BASE = 10000.0
WSC = 16.0          # host-side weight prescale into fp8 normal range
LN_WSC = float(np.log(WSC))


class Cfg:
    def __init__(self, B=2, S=2048, D=1024, H=16, L=4):
        self.B, self.S, self.D, self.H, self.L = B, S, D, H, L
        self.HD = D // H
        assert self.HD == 64
        self.T = B * S // N_CORES            # tokens per core (Tb from each batch)
        self.Tb = self.T // B
        assert self.T * N_CORES == B * S and self.Tb % P == 0
        self.DC = D // P                     # d-chunks of 128
        assert D % 512 == 0
        self.QT = min(512, S)                # attention q-tile width
        assert S % self.QT == 0 and self.QT % self.Tb == 0 or self.Tb % self.QT == 0
        self.NQ = S // self.QT               # global q tiles
        self.HPC = H // 8                    # head-pairs per core
        assert H % 8 == 0
        self.D4 = D // 4                     # head-quad rows per a2a shard
        self.OSH = self.DC // 4              # o-chunks per head-quad
        self.VW = min(512, D)
        self.NVW = D // self.VW
        self.NCT = 4 * D // P                # weight c-tiles (128 wide)


DEFAULT_CFG = Cfg()


def build(cfg=DEFAULT_CFG, n_layers=None):
    B, S, D, H = cfg.B, cfg.S, cfg.D, cfg.H
    L = cfg.L if n_layers is None else n_layers
    T, DC, QT, NQ, HPC = cfg.T, cfg.DC, cfg.QT, cfg.NQ, cfg.HPC
    HD, D4, OSH, VW, NVW = cfg.HD, cfg.D4, cfg.OSH, cfg.VW, cfg.NVW
    Tb = cfg.Tb
    TPb = Tb // P
    HD2 = HD // 2
    scale = float(1.0 / np.sqrt(HD))
    TP = T // P                              # token chunks of 128
    NPR = DC // 2                            # DoubleRow contraction pairs

    nc = bacc.Bacc("TRN2", target_bir_lowering=False, debug=False,
                   num_devices=N_CORES)
    prev_cc = [None]

    def chain_cc(cc):
        # pin collective issue order (the Tile scheduler otherwise reorders
        # them, serializing a later collective ahead of an earlier one)
        if prev_cc[0] is not None:
            bass._add_dep_helper(cc.ins, prev_cc[0].ins, sync=True,
                                 reason="collective issue order")
        prev_cc[0] = cc
        return cc

    def _raw(h):
        return h.ins if hasattr(h, "ins") else h

    def fence(later, earliers):
        # forced ordering: keep same-queue instructions in ready-time order so
        # queue-head waits don't block earlier-ready work (sync=True: the
        # scheduler ignores nosync edges)
        for e in earliers:
            if e is not None and later is not None:
                bass._add_dep_helper(_raw(later), _raw(e), sync=True,
                                     reason="queue order fence")

    # ---------------- parameters ----------------
    xT = nc.declare_dram_parameter("xT", [P, DC, T], F32, isOutput=False)
    tda = nc.declare_dram_parameter("tda", [1, S], F32, isOutput=False)
    pida = nc.declare_dram_parameter("pida", [1, S], F32, isOutput=False)
    invf = nc.declare_dram_parameter("invf", [HD2, 1], F32, isOutput=False)
    # host pre-tiled weights: [L, NCT, DC, P, 128] / [L, D//P, DC, P, 128]
    uvqk_w = nc.declare_dram_parameter("uvqk_w", [L, cfg.NCT, P, DC, P], BF16, isOutput=False)
    out_w = nc.declare_dram_parameter("out_w", [L, DC, P, DC, P], BF16, isOutput=False)
    uvqk_b = nc.declare_dram_parameter("uvqk_b", [L, 4 * D], F32, isOutput=False)
    gate_w = nc.declare_dram_parameter("gate_w", [L, D], F32, isOutput=False)
    out_b = nc.declare_dram_parameter("out_b", [L, D], F32, isOutput=False)
    in_nw = nc.declare_dram_parameter("in_nw", [D], F32, isOutput=False)
    last_nw = nc.declare_dram_parameter("last_nw", [D], F32, isOutput=False)
    out_ext = nc.declare_dram_parameter("out", [DC, P, T], F32, isOutput=True)

    with tile.TileContext(nc) as tc:
        with (
            tc.tile_pool(name="const", bufs=1) as cpool,
            tc.tile_pool(name="persist", bufs=1) as ppool,
            tc.tile_pool(name="setup", bufs=1) as spool,
            tc.tile_pool(name="big", bufs=1) as bpool,
            tc.tile_pool(name="wpool", bufs=int(os.environ.get("WPOOL", "4"))) as wpool,
            tc.tile_pool(name="wvpool", bufs=int(os.environ.get("WVP", "2"))) as wvpool,
            tc.tile_pool(name="work", bufs=2) as work,
            tc.tile_pool(name="wtile", bufs=int(os.environ.get("WTL", "3"))) as wtl,
            tc.tile_pool(name="kv", bufs=int(os.environ.get("KVB", "1"))) as kvpool,
            tc.tile_pool(name="ups", bufs=3, space="PSUM") as ups,
            tc.tile_pool(name="aps", bufs=1, space="PSUM") as aps,
            tc.tile_pool(name="ssps", bufs=1, space="PSUM") as ssps,
            tc.tile_pool(name="dram", bufs=1, space="DRAM") as dram,
        ):
            # ---------------- constants ----------------
            ones_bf = cpool.tile([P, 1], BF16)
            nc.vector.memset(ones_bf[:], 1.0)
            eps_t = cpool.tile([P, 1], F32)
            nc.vector.memset(eps_t[:], EPS)
            nlw_t = cpool.tile([P, 1], F32)
            nc.vector.memset(nlw_t[:], -LN_WSC)

            b_u = ppool.tile([P, L, DC], F32)
            b_q = ppool.tile([P, L, DC], F32)   # host-prescaled x16
            b_k = ppool.tile([P, L, DC], F32)   # host-prescaled x16
            g_w = ppool.tile([P, L, DC], F32)
            o_b = ppool.tile([P, L, DC], F32)
            for ll in range(L):
                nc.scalar.dma_start(b_u[:, ll, :], uvqk_b[ll, 0:D].rearrange("(o p) -> p o", p=P))
                nc.scalar.dma_start(b_q[:, ll, :], uvqk_b[ll, 2 * D:3 * D].rearrange("(o p) -> p o", p=P))
                nc.scalar.dma_start(b_k[:, ll, :], uvqk_b[ll, 3 * D:4 * D].rearrange("(o p) -> p o", p=P))
                nc.scalar.dma_start(g_w[:, ll, :], gate_w[ll].rearrange("(o p) -> p o", p=P))
                nc.scalar.dma_start(o_b[:, ll, :], out_b[ll].rearrange("(o p) -> p o", p=P))
            in_nw_t = ppool.tile([P, DC], F32)
            last_nw_t = ppool.tile([P, DC], F32)
            nc.scalar.dma_start(in_nw_t[:], in_nw.rearrange("(o p) -> p o", p=P))
            nc.scalar.dma_start(last_nw_t[:], last_nw.rearrange("(o p) -> p o", p=P))
            bv_row = ppool.tile([1, L, D], F32)
            nc.scalar.dma_start(bv_row[:], uvqk_b[None, :, D:2 * D])

            cosF = ppool.tile([P, S], BF16)
            sinF = ppool.tile([P, S], BF16)

            if True:  # rope-table build (eager)
                # ------- rope tables over the receiver's full attention axis -----
                # (scratch-reusing chain: X/Y/Z are [HD2, S]; A/B/C are [1, S])
                t1a = spool.tile([1, S], F32)
                t32x = spool.tile([HD2, S], F32)
                t32y = spool.tile([HD2, S], F32)
                t32z = spool.tile([HD2, S], F32)
                t1b = t32y[0:1, :]
                invf_t = cpool.tile([HD2, 1], F32)
                nc.sync.dma_start(t1a[:], tda[:, :])
                nc.sync.dma_start(invf_t[:], invf[:, :])
                nc.scalar.activation(t1b, t1a[:], AF.Ln, bias=1.0)
                nc.sync.dma_start(t1a[:], pida[:, :])
                nc.vector.scalar_tensor_tensor(t1a[:], t1b, 0.1, t1a[:],
                                                       op0=ALU.mult, op1=ALU.add)
                nc.gpsimd.partition_broadcast(t32x[:], t1a[:], channels=HD2)
                nc.vector.tensor_scalar(t32x[:], t32x[:], invf_t[:], None, op0=ALU.mult)
                nc.vector.tensor_scalar(t32y[:], t32x[:], float(1.0 / (2 * np.pi)), None, op0=ALU.mult)
                nc.vector.tensor_scalar(t32y[:], t32y[:], MAGIC, MAGIC, op0=ALU.add, op1=ALU.subtract)
                nc.vector.cody_waite_cascade(t32z[:], t32x[:], t32y[:], CW_C1, CW_C2, CW_C3)
                nc.vector.add_range_wrap(t32y[:], t32z[:], float(np.pi / 2),
                                                 float(np.pi), float(2 * np.pi))
                cs32 = t32x
                sn32 = t32z
                nc.scalar.activation(cs32[:], t32y[:], AF.Sin)
                nc.scalar.activation(sn32[:], t32z[:], AF.Sin)
                for hh in range(2):
                    b0 = hh * 64
                    nc.vector.tensor_copy(cosF[b0:b0 + HD2, :], cs32[:])
                    nc.vector.tensor_copy(cosF[b0 + HD2:b0 + HD, :], cs32[:])
                    nc.vector.tensor_scalar(sinF[b0:b0 + HD2, :], sn32[:], -1.0, None, op0=ALU.mult)
                    nc.vector.tensor_copy(sinF[b0 + HD2:b0 + HD, :], sn32[:])


            # ---------------- helpers ----------------
            def rms_scale_from(src_tile, src_fp8=True):
                """[1,T] inv-rms over D of a [P, DC, T] tile -> bcast [P, T] f32."""
                ss_ps = ssps.tile([1, T], F32, tag="ssq")
                for o in range(DC):
                    sq = work.tile([P, T], BF16, tag="sq")
                    nc.vector.tensor_tensor(sq[:], src_tile[:, o, :], src_tile[:, o, :], op=ALU.mult)
                    nc.tensor.matmul(ss_ps[:], ones_bf[:], sq[:],
                                     start=(o == 0), stop=(o == DC - 1))
                lnm = work.tile([1, T], F32, tag="lnm")
                nc.scalar.activation(lnm[:], ss_ps[:], AF.Ln, scale=1.0 / D, bias=eps_t[:1])
                sc = work.tile([1, T], F32, tag="rms")
                nc.scalar.activation(sc[:], lnm[:], AF.Exp, scale=-0.5)
                sc_bc = work.tile([P, T], F32, tag="rmsbc")
                nc.gpsimd.partition_broadcast(sc_bc[:], sc[:], channels=P)
                return sc_bc

            # zero-padded interleaved V for full-width DoubleRow AV:
            # vstk[p, g, j, d] nonzero only at (g=0, d<64) and (g=1, d>=64)
            vstk = [ppool.tile([P, 2, S // P, P], FP8, name=f"vstk{i}")
                    for i in range(2)]
            for i in range(2):
                nc.vector.memset(vstk[i][:], 0.0)

            # ---------------- h0 = rms_norm(x, in_norm_w) ----------------
            h = ppool.tile([P, DC, T], F32)
            h_bf = ppool.tile([P, DC, T], BF16)
            for o in range(DC):
                nc.sync.dma_start(h[:, o, :], xT[:, o, :])
            ss_ps = ssps.tile([1, T], F32, tag="ssq")
            for o in range(DC):
                sq = work.tile([P, T], BF16, tag="sq")
                nc.vector.tensor_tensor(sq[:], h[:, o, :], h[:, o, :], op=ALU.mult)
                nc.tensor.matmul(ss_ps[:], ones_bf[:], sq[:],
                                 start=(o == 0), stop=(o == DC - 1))
            lnm = work.tile([1, T], F32, tag="lnm")
            nc.scalar.activation(lnm[:], ss_ps[:], AF.Ln, scale=1.0 / D, bias=eps_t[:1])
            sc0 = work.tile([1, T], F32, tag="rms")
            nc.scalar.activation(sc0[:], lnm[:], AF.Exp, scale=-0.5)
            sc_bc = work.tile([P, T], F32, tag="rmsbc")
            nc.gpsimd.partition_broadcast(sc_bc[:], sc0[:], channels=P)
            for o in range(DC):
                nc.vector.scalar_tensor_tensor(h[:, o, :], h[:, o, :],
                                               in_nw_t[:, o:o + 1], sc_bc[:],
                                               op0=ALU.mult, op1=ALU.mult)
                nc.scalar.copy(h_bf[:, o, :], h[:, o, :])

            # ---------------- a2a dram buffers (flat shards, per half) -------
            RB = P * Tb
            SH1 = 5 * RB
            a2a_in1 = [dram.tile([8, SH1], FP8, name=f"a2a_in1_{i}") for i in range(2)]
            a2a_out1 = [dram.tile([8, SH1], FP8, name=f"a2a_out1_{i}") for i in range(2)]
            a2a_in2 = [dram.tile([8, RB], BF16, name=f"a2a_in2_{i}") for i in range(2)]
            a2a_out2 = [dram.tile([8, RB], BF16, name=f"a2a_out2_{i}") for i in range(2)]

            for l in range(L):
                # ---------------- uvqk projection + a2a1, per head-pair half --
                uT = ppool.tile([P, DC, T], BF16, tag="uT")
                bv_bc = spool.tile([P, D], F32, tag="bvbc")
                nc.gpsimd.partition_broadcast(bv_bc[:], bv_row[:, l, :], channels=P)

                uvqk_dve = []
                uvqk_mm = []
                for hf in range(2):
                    # per-half staging (packed out before the next half lands)
                    qT = ppool.tile([P, 4, T], BF16, tag="qT")   # [p, quad, t]
                    kT = ppool.tile([P, 4, T], BF16, tag="kT")
                    # v token-major, a2a-payload order: [p, quad, ba, tb-chunk, c]
                    vtm = ppool.tile([P, 4, 2, TPb, P], FP8, tag="vtm")
                    # q,k chunks of this half (+rope), interleaved per quad.
                    # rot_half comes from a second matmul against host-rotated
                    # weights; bias-adds ride the Act engine (idle in proj).
                    for qd in range(4):
                        ct = 2 * qd + hf
                        for cbase, dest, bias in (
                                (2 * DC, qT, b_q), (3 * DC, kT, b_k)):
                            wt = wpool.tile([P, DC, P], BF16, tag="w_uqk")
                            nc.sync.dma_start(wt[:], uvqk_w[l, cbase + ct])
                            psb = ups.tile([P, 1024], F32, tag="ps")
                            ps = psb[:, 0:T]
                            for dc in range(DC):
                                uvqk_mm.append(nc.tensor.matmul(
                                    ps, wt[:, dc, :], h_bf[:, dc, :],
                                    start=(dc == 0), stop=(dc == DC - 1)))
                            uvqk_dve.append(nc.scalar.activation(
                                dest[:, qd, :], ps, AF.Identity,
                                bias=bias[:, l, ct:ct + 1]))
                    # v half: host-permuted weight tiles DC+4*hf .. DC+4*hf+3
                    vt_w = wvpool.tile([P, DC, VW], BF16, tag="w_v")
                    nc.sync.dma_start(
                        vt_w[:].rearrange("p d (a c) -> p d a c", c=P),
                        uvqk_w[l, DC + 4 * hf:DC + 4 * (hf + 1)]
                        .rearrange("a p d c -> p d a c"))
                    for tt in range(TP):
                        ba, aa = divmod(tt, TPb)
                        psb = ups.tile([P, 1024], F32, tag="ps")
                        ps = psb[:, 0:VW]
                        for dc in range(DC):
                            uvqk_mm.append(nc.tensor.matmul(
                                ps, h_bf[:, dc, tt * P:(tt + 1) * P],
                                vt_w[:, dc, :],
                                start=(dc == 0), stop=(dc == DC - 1)))
                        uvqk_dve.append(nc.vector.tensor_tensor(
                            vtm[:, :, ba, aa, :],
                            ps.rearrange("p (q c) -> p q c", c=P),
                            bv_bc[:, hf * VW:(hf + 1) * VW]
                            .rearrange("p (q c) -> p q c", c=P), op=ALU.add))
                    # pack (2 DMAs per tensor) + collective for this half
                    qTb = qT[:].bitcast(FP8)
                    kTb = kT[:].bitcast(FP8)
                    for ba in range(2):
                        jsl = slice(ba * 4, (ba + 1) * 4)
                        tslb = slice(ba * 2 * Tb, (ba + 1) * 2 * Tb)
                        nc.gpsimd.dma_start(
                            a2a_in1[hf][jsl, 0:2 * RB]
                            .rearrange("j (p t) -> p j t", p=P),
                            qTb[:, :, tslb])
                        nc.gpsimd.dma_start(
                            a2a_in1[hf][jsl, 2 * RB:4 * RB]
                            .rearrange("j (p t) -> p j t", p=P),
                            kTb[:, :, tslb])
                        nc.gpsimd.dma_start(
                            a2a_in1[hf][jsl, 4 * RB:5 * RB]
                            .rearrange("j (p a c) -> p j a c", p=P, c=P),
                            vtm[:, :, ba, :, :])
                    chain_cc(nc.gpsimd.collective_compute(
                        "AllToAll", ALU.bypass, replica_groups=GROUPS,
                        ins=[a2a_in1[hf][:].opt()], outs=[a2a_out1[hf][:].opt()]))

                def emit_u(uct):
                    wtu = wpool.tile([P, DC, P], BF16, tag="w_uqk")
                    nc.sync.dma_start(wtu[:], uvqk_w[l, uct])
                    psb_u = ups.tile([P, 1024], F32, tag="ps")
                    ps_u = psb_u[:, 0:T]
                    for dc in range(DC):
                        uvqk_mm.append(nc.tensor.matmul(
                            ps_u, wtu[:, dc, :], h_bf[:, dc, :],
                            start=(dc == 0), stop=(dc == DC - 1)))
                    nc.scalar.activation(uT[:, uct, :], ps_u, AF.Silu,
                                         bias=b_u[:, l, uct:uct + 1])

                # half the u projection fills the a2a1 flight; the rest rides
                # behind attn-h0's PE stream (u is needed only at gating)
                for uct in range(4):
                    emit_u(uct)

                # ---------------- attention (head-parallel, per half) ---------
                # at2[p, hf, quad, t]: d-chunk (2*quad+hf) of attn (gating input)
                at2 = bpool.tile([P, 2, 4, T], BF16, tag="at")
                prev_half_dve = None
                prev_half_mm = None
                last_attn_mm = None
                last_attnT_copy = None
                last_silu = [None]
                last_in2_pack = [None]
                for hf in range(2):
                    q_hp = kvpool.tile([P, S // Tb, Tb], BF16, tag="q_hp")
                    k_hp = kvpool.tile([P, S // Tb, Tb], BF16, tag="k_hp")
                    v_st = vstk[hf]
                    nc.scalar.dma_start(
                        q_hp[:].bitcast(FP8),
                        a2a_out1[hf][:, 0:2 * RB].rearrange("r (p t) -> p r t", p=P))
                    nc.sync.dma_start(
                        k_hp[:].bitcast(FP8),
                        a2a_out1[hf][:, 2 * RB:4 * RB].rearrange("r (p t) -> p r t", p=P))
                    vr = v_st[:].rearrange("p g (r a) d -> p g r a d", a=TPb)
                    for g in range(2):
                        for a in range(TPb):
                            nc.sync.dma_start(
                                vr[:, g, :, a, g * 64:(g + 1) * 64].opt(),
                                a2a_out1[hf][:, 4 * RB:5 * RB]
                                .rearrange("r (p a c) -> p r a c", p=P, c=P)
                                [:, :, a, g * 64:(g + 1) * 64].opt())
                    # receiver-side rope, per 256-token block (DVE rides
                    # under the Act-bound silu stream)
                    NAB = S // Tb
                    rope_order = [("k", 0), ("q", 0), ("q", 1), ("k", 1),
                                  ("k", 2), ("k", 3), ("q", 2), ("q", 3),
                                  ("k", 4), ("k", 5), ("q", 4), ("q", 5),
                                  ("k", 6), ("k", 7), ("q", 6), ("q", 7)]
                    for which, ab in rope_order:
                        csl = slice(ab * Tb, (ab + 1) * Tb)
                        for src in ((k_hp,) if which == "k" else (q_hp,)):
                            blk = src[:, ab, :]
                            sw = work.tile([P, Tb], BF16, tag="rxswap")
                            for hh in range(2):
                                b0 = hh * 64
                                nc.vector.tensor_copy(sw[b0:b0 + HD2, :],
                                                      blk[b0 + HD2:b0 + HD, :])
                                nc.vector.tensor_copy(sw[b0 + HD2:b0 + HD, :],
                                                      blk[b0:b0 + HD2, :])
                            t1 = work.tile([P, Tb], BF16, tag="rxt1")
                            nc.vector.tensor_tensor(t1[:], blk, cosF[:, csl], op=ALU.mult)
                            t2 = work.tile([P, Tb], BF16, tag="rxt2")
                            nc.vector.tensor_tensor(t2[:], sw[:], sinF[:, csl], op=ALU.mult)
                            nc.vector.tensor_tensor(blk, t1[:], t2[:], op=ALU.add)
                    attnT = bpool.tile([P, NQ, QT], BF16, tag="attnT")
                    k_fl = k_hp[:].rearrange("p a t -> p (a t)")
                    q_fl = q_hp[:].rearrange("p a t -> p (a t)")
                    # software-pipelined: emit scores LA stages ahead of the
                    # (DoubleRow, padded-V) av matmuls
                    stages = [(qt, ji, (qt + 1) * QT // P)
                              for qt in range(NQ)
                              for ji in range((qt + 1) * QT // P)]
                    LA = int(os.environ.get("LA", "3"))
                    w_ts = {}
                    a_ps_map = {}

                    def emit_front(i, hf):
                        nonlocal prev_half_mm, uvqk_mm, prev_half_dve
                        nonlocal uvqk_dve, last_attn_mm
                        qt, ji, njt = stages[i]
                        w_t = wtl.tile([P, 2, QT], FP8, tag="wt", name="w_t")
                        s_ps = ups.tile([P, 1024], F32, tag="ps")
                        for hh in range(2):
                            smm = nc.tensor.matmul(
                                s_ps[:, hh * 512:hh * 512 + QT],
                                k_fl[hh * 64:(hh + 1) * 64, ji * P:(ji + 1) * P],
                                q_fl[hh * 64:(hh + 1) * 64, qt * QT:(qt + 1) * QT],
                                start=True, stop=True,
                                tile_position=(hh * 64, 0))
                            if prev_half_mm is not None:
                                # attn-h1 PE work strictly after attn-h0's
                                fence(smm, [prev_half_mm])
                                prev_half_mm = None
                            if uvqk_mm and FENCE_SCORE:
                                fence(smm, uvqk_mm)
                            if uvqk_mm:
                                uvqk_mm = []
                            last_attn_mm = smm
                        sview = s_ps[:].rearrange("p (h x) -> p h x", h=2)[:, :, 0:QT]
                        last_silu[0] = nc.scalar.activation(
                            w_t[:], sview, AF.Silu, scale=scale)
                        dq = qt * QT - ji * P
                        if dq < P:  # diagonal-crossing j tile: mask on Pool
                            mi = nc.gpsimd.affine_select(
                                w_t[:], w_t[:],
                                pattern=[[0, 2], [1, QT]],
                                compare_op=ALU.is_ge, fill=0.0,
                                base=dq, channel_multiplier=-1)
                            if prev_half_dve is not None:
                                fence(mi, [prev_half_dve])
                                prev_half_dve = None
                            if uvqk_dve and FENCE_DVE:
                                fence(mi, uvqk_dve)
                            if uvqk_dve:
                                uvqk_dve = []
                        w_ts[i] = w_t

                    def emit_back(i, hf):
                        nonlocal last_attn_mm, prev_half_dve, last_attnT_copy
                        qt, ji, njt = stages[i]
                        if ji == 0:
                            a_ps_map[qt] = aps.tile([P, QT], F32, tag="aps",
                                                    name="a_ps")
                        a_ps = a_ps_map[qt]
                        w_t = w_ts.pop(i)
                        last_attn_mm = nc.tensor.matmul(
                            a_ps[:], v_st[:, :, ji, :], w_t[:],
                            start=(ji == 0), stop=(ji == njt - 1),
                            perf_mode=DR)
                        if ji == njt - 1:
                            atc = nc.vector.tensor_copy(attnT[:, qt, :], a_ps[:])
                            last_in2_pack[0] = nc.sync.dma_start(
                                a2a_in2[hf][2 * qt:2 * (qt + 1), :]
                                .rearrange("r (p t) -> p r t", p=P),
                                attnT[:, qt, :].rearrange("p (r t) -> p r t", t=Tb))
                            if qt == NQ - 1:
                                if hf == 0:
                                    prev_half_dve = atc
                                else:
                                    last_attnT_copy = atc

                    for i in range(len(stages)):
                        emit_front(i, hf)
                        if i >= LA:
                            emit_back(i - LA, hf)
                    for i in range(len(stages) - LA, len(stages)):
                        emit_back(i, hf)
                    chain_cc(nc.gpsimd.collective_compute(
                        "AllToAll", ALU.bypass, replica_groups=GROUPS,
                        ins=[a2a_in2[hf][:].opt()], outs=[a2a_out2[hf][:].opt()]))
                    if hf == 0:
                        prev_half_mm = last_attn_mm
                        for uct in range(4, DC):
                            emit_u(uct)

                # ---------------- gated rms + out proj (token-local) ----------
                # h0's attn arrives one collective early: unpack it on SP
                # (fenced behind the last attnT pack) so its ssq/gating overlap
                # attn-h1; h1's unpack stays on Act behind the last silu.
                for hf2 in range(2):
                    for ba in range(2):
                        if hf2 == 0:
                            atd = nc.sync.dma_start(
                                at2[:, hf2, :, ba * Tb:(ba + 1) * Tb],
                                a2a_out2[hf2][ba * 4:(ba + 1) * 4]
                                .rearrange("r (p t) -> p r t", p=P))
                            fence(atd, [last_in2_pack[0]])
                        else:
                            atd = nc.scalar.dma_start(
                                at2[:, hf2, :, ba * Tb:(ba + 1) * Tb],
                                a2a_out2[hf2][ba * 4:(ba + 1) * 4]
                                .rearrange("r (p t) -> p r t", p=P))
                            fence(atd, [last_silu[0]])
                # inv-rms over D (per token) from at2; ssq on PE via ones-matmul
                # (half-0 chunks first + h0 gating before h1's sq, so DVE never
                # head-blocks on the h1 collective)
                oorder = [0, 2, 4, 6, 1, 3, 5, 7]
                ss_ps = ssps.tile([1, T], F32, tag="ssq")
                gate_half = {0: [], 1: []}
                for i, o in enumerate(oorder):
                    sq = work.tile([P, T], BF16, tag="sq")
                    src = at2[:, o % 2, o // 2, :]
                    sqi = nc.vector.tensor_tensor(sq[:], src, src, op=ALU.mult)
                    if i == 0:
                        fence(sqi, [last_attnT_copy])
                    qmm = nc.tensor.matmul(ss_ps[:], ones_bf[:], sq[:],
                                           start=(i == 0), stop=(i == DC - 1))
                    if i == 0 and FENCE_SSQ:
                        fence(qmm, [last_attn_mm])
                    if i == 3:
                        # gate h0 chunks while h1's attn output is still in flight
                        for o2 in (0, 2, 4, 6):
                            gate_half[0].append(nc.vector.scalar_tensor_tensor(
                                at2[:, o2 % 2, o2 // 2, :],
                                at2[:, o2 % 2, o2 // 2, :],
                                g_w[:, l, o2:o2 + 1], uT[:, o2, :],
                                op0=ALU.mult, op1=ALU.mult))
                lnm = work.tile([1, T], F32, tag="lnm")
                nc.scalar.activation(lnm[:], ss_ps[:], AF.Ln, scale=1.0 / D, bias=eps_t[:1])
                sc = work.tile([1, T], F32, tag="rms")
                nc.scalar.activation(sc[:], lnm[:], AF.Exp, scale=-0.5)
                sc_bc = work.tile([P, T], F32, tag="rmsbc")
                nc.gpsimd.partition_broadcast(sc_bc[:], sc[:], channels=P)
                for o in (1, 3, 5, 7):
                    nc.vector.scalar_tensor_tensor(at2[:, o % 2, o // 2, :],
                                                   at2[:, o % 2, o // 2, :],
                                                   g_w[:, l, o:o + 1], uT[:, o, :],
                                                   op0=ALU.mult, op1=ALU.mult)
                for et in range(DC):
                    wt = wpool.tile([P, DC, P], BF16, tag="w_o")
                    nc.sync.dma_start(wt[:], out_w[l, et])
                    psb = ups.tile([P, 1024], F32, tag="ps")
                    ps = psb[:, 0:T]
                    for i, dc in enumerate(oorder):
                        nc.tensor.matmul(ps, wt[:, dc, :],
                                         at2[:, dc % 2, dc // 2, :],
                                         start=(i == 0), stop=(i == DC - 1))
                    otmp = work.tile([P, T], F32, tag="xchunk")
                    nc.vector.tensor_tensor(otmp[:], ps, sc_bc[:], op=ALU.mult)
                    nc.vector.scalar_tensor_tensor(h[:, et, :], otmp[:], o_b[:, l, et:et + 1],
                                                   h[:, et, :], op0=ALU.add, op1=ALU.add)
                    nc.scalar.copy(h_bf[:, et, :], h[:, et, :])

            # ---------------- final norm ----------------
            sc_bc = rms_scale_from(h_bf)
            for o in range(DC):
                of = work.tile([P, T], F32, tag="of")
                nc.vector.scalar_tensor_tensor(of[:], h[:, o, :], last_nw_t[:, o:o + 1],
                                               sc_bc[:], op0=ALU.mult, op1=ALU.mult)
                nc.sync.dma_start(out_ext[o, :, :], of[:])

    nc.compile()
    return nc


def shard_inputs(inputs, cfg=DEFAULT_CFG):
    B, S, D, H, L = cfg.B, cfg.S, cfg.D, cfg.H, cfg.L
    T, DC, P_ = cfg.T, cfg.DC, P
    x = np.asarray(inputs["x"], dtype=np.float32)
    tdel = np.asarray(inputs["time_deltas"], dtype=np.float32)
    pids = np.asarray(inputs["position_ids"]).astype(np.int32)
    bfd = ml_dtypes.bfloat16
    # weights pre-tiled: [L, NCT, DC, P, 128]; v col-tiles permuted so each
    # head-pair half's 512 cols are contiguous: tile DC+4*hf+hq = heads (hq,hf)
    uw = np.asarray(inputs["uvqk_w"], dtype=np.float32)
    uw = uw.reshape(L, DC, P_, cfg.NCT, P_).transpose(0, 3, 2, 1, 4)
    vperm = [8 + 2 * hq + hf for hf in (0, 1) for hq in range(4)]
    tile_order = list(range(8)) + vperm + list(range(16, 32))
    uw = np.ascontiguousarray(uw[:, tile_order])
    uw = uw.astype(bfd)
    ow = np.asarray(inputs["out_w"], dtype=np.float32).astype(bfd)
    ow = np.ascontiguousarray(
        ow.reshape(L, DC, P_, DC, P_).transpose(0, 3, 2, 1, 4))
    ub = np.asarray(inputs["uvqk_b"], dtype=np.float32).copy()
    vdims = np.concatenate([np.arange(p * P_, (p + 1) * P_) for p in vperm]) - 8 * P_
    ub[:, D:2 * D] = ub[:, D + vdims]
    ub = np.ascontiguousarray(ub)
    gw = np.ascontiguousarray(np.asarray(inputs["gate_w"], dtype=np.float32))
    ob = np.ascontiguousarray(np.asarray(inputs["out_b"], dtype=np.float32))
    inw = np.ascontiguousarray(np.asarray(inputs["in_norm_w"], dtype=np.float32))
    lnw = np.ascontiguousarray(np.asarray(inputs["last_norm_w"], dtype=np.float32))
    invf = (1.0 / (ROPE_BASE ** (np.arange(0, cfg.HD, 2, dtype=np.float32) / cfg.HD))
            ).astype(np.float32).reshape(-1, 1)
    Tb = cfg.Tb
    in_maps = []
    for c in range(N_CORES):
        sl = slice(c * Tb, (c + 1) * Tb)
        xc = np.concatenate([x[b, sl, :] for b in range(B)], axis=0)  # [T, D]
        xTc = np.ascontiguousarray(
            xc.T.reshape(DC, P_, T).transpose(1, 0, 2))  # [P, DC, T]
        tdc = np.concatenate([tdel[b, sl] for b in range(B)])
        pidc = np.concatenate([pids[b, sl] for b in range(B)])
        ab = c // 4
        in_maps.append({
            "xT": xTc,
            "tda": np.ascontiguousarray(tdel[ab].reshape(1, S)),
            "pida": np.ascontiguousarray(pids[ab].reshape(1, S).astype(np.float32)),
            "invf": invf,
            "uvqk_w": uw, "uvqk_b": ub, "gate_w": gw,
            "out_w": ow, "out_b": ob, "in_nw": inw, "last_nw": lnw,
        })
    return in_maps


def unshard_output(results, cfg=DEFAULT_CFG):
    B, S, D, T, Tb = cfg.B, cfg.S, cfg.D, cfg.T, cfg.Tb
    out = np.empty((B, S, D), dtype=np.float32)
    for c in range(N_CORES):
        oc = np.asarray(results[c]["out"], dtype=np.float32).reshape(D, T).T  # [T, D]
        for b in range(B):
            out[b, c * Tb:(c + 1) * Tb, :] = oc[b * Tb:(b + 1) * Tb]
    return out


_NC_CACHE = {}


def _get_nc(cfg=DEFAULT_CFG):
    key = (cfg.B, cfg.S, cfg.D, cfg.H, cfg.L)
    if key not in _NC_CACHE:
        _NC_CACHE[key] = build(cfg)
    return _NC_CACHE[key]


def kernel(**inputs):
    cfg = DEFAULT_CFG
    nc = _get_nc(cfg)
    in_maps = shard_inputs(inputs, cfg)
    r = run_bass_kernel_spmd(nc, in_maps, core_ids=list(range(N_CORES)), trace=False)
    return unshard_output(r.results, cfg)
